# revision 36
# baseline (speedup 1.0000x reference)
"""Trainium2 Bass kernel for nn_MixedAttnHeadEmbed (mixed-head-config attention).

v2: bf16 end-to-end with [q,d]-layout outputs.

Math (per batch b): two attention configs share q_m/k_m/v_m [B,T,2048]:
  A: h=8  heads, d_max=256, mixing e in {1024,2048} -> d in {128,256}, w0,w1
  B: h=16 heads, d_max=128, mixing e in {1024,2048} -> d in {64,128},  w2,w3
Sharding: 8 cores = 4 batches x 2 shards; shard s owns A-heads [4s,4s+4) and
B-heads [8s,8s+8) -> output cols [1024s, 1024s+1024), written [T, 1024].

Device design notes:
 - Everything bf16 on SBUF (DVE 2x/4x fast modes, full-rate PE, half DMA);
   PSUM f32 only for matmul accumulation.
 - Weight folding moves all mix scalars into host-precomputed rope tables:
     qmA = fA^2 w0 (w0 P(r1q) + w1 R2q),   kmA = P(r1k) + (w1/w0) R2k
     qmB = fB^2 w3 (w3 R128(x) + w2 P(r64q)), kmB = r1k[g] + (w2/w3) P(r64k)
   where r1 = unscaled rope128 (k-side shared between A and B).
 - Rope rotations are free: sigma-permuted DATA copies ship from host, so
   rope(x) = x*c + xs*s with signed math-order sin tables (3 TT per tile).
 - Scores computed transposed sT[k,q] per k-chunk; causal mask applied ON THE
   PE (strict-tri(-1e9) @ I accumulated into the diagonal block); exp on ACT
   (one inst per chunk) into an SBUF bf16 pt [128, 8, T].
 - Phase 2 emits y in [q-part, d-free] via pt-stationary matmuls with an
   AUGMENTED V (ones column) so the softmax denominator is the last y column;
   normalize = tensor_scalar divide by that per-partition column (no
   reciprocal, no transposes, output lands in natural [T, H] layout).
 - A static engine balancer routes elementwise work DVE/Pool by modeled cost.
"""

import math
from contextlib import ExitStack
from dataclasses import dataclass

import numpy as np
import ml_dtypes

import concourse.bass as bass
import concourse.mybir as mybir
import concourse.tile as tile
from concourse import bacc

F32 = mybir.dt.float32
BF = mybir.dt.bfloat16
NPBF = ml_dtypes.bfloat16
NEG = -1e9
P = 128
T = 1024
TK = T // P


@dataclass(frozen=True)
class KCfg:
    pass


FULL = KCfg()

mult = mybir.AluOpType.mult
add = mybir.AluOpType.add
sub = mybir.AluOpType.subtract
div = mybir.AluOpType.divide
Exp = mybir.ActivationFunctionType.Exp


class _Pick:
    """Cost-model-accurate static balancer for DVE / Pool elementwise ops."""

    def __init__(self, nc):
        self.nc = nc
        self.load = {"dve": 0.0, "pool": 0.0}

    def _choose(self, cd, cp, psum=False):
        # GPSIMD cannot access PSUM (BIR verifier rule) -> DVE only then
        if psum or self.load["dve"] + cd <= self.load["pool"] + cp:
            self.load["dve"] += cd
            return self.nc.vector
        self.load["pool"] += cp
        return self.nc.gpsimd

    def tt(self, out, a, b, op, free, psum=False):
        cd = (1.0417 * free + 125) if psum else (0.52 * free + 60)
        eng = self._choose(cd, 0.8333 * free + 120, psum)
        eng.tensor_tensor(out, a, b, op)

    def tsp(self, out, a, scal, op, free, psum=False):
        cd = (1.0417 * free + 125) if psum else (0.26 * free + 60)
        eng = self._choose(cd, 0.8333 * free + 120, psum)
        eng.tensor_scalar(out=out, in0=a, scalar1=scal, scalar2=None, op0=op)

    def stt(self, out, a, scal, b, op0, op1, free, psum=False):
        cd = (1.0417 * free + 125) if psum else (1.0417 * free + 60)
        eng = self._choose(cd, 0.8333 * free + 120, psum)
        eng.scalar_tensor_tensor(out=out, in0=a, scalar=scal, in1=b,
                                 op0=op0, op1=op1)

    def cp(self, dst, src, free, psum=False):
        cd = (1.0417 * free + 125) if psum else (0.26 * free + 60)
        eng = self._choose(cd, 0.8333 * free + 120, psum)
        eng.tensor_copy(dst, src)


def build_program(cfg: KCfg = FULL):
    nc = bacc.Bacc("TRN2", target_bir_lowering=False)

    def dram(name, shape, dt=BF, out=False):
        return nc.declare_dram_parameter(name, list(shape), dt, isOutput=out)

    D = {
        # per-group: r1in ch (qa1 qa1s ka1 ka1s); grp ch 0-1 qa2, 2-3 ka2,
        # 4-5 d64q pair (rows 0:64 sigma32 | 64:128 raw), 6 d64k
        "r1in": dram("r1in", (4, 4, P, T)),
        "grp": dram("grp", (4, 7, P, T)),
        "tabr1": dram("tabr1", (4, P, T)),     # c1q s1q c1 s1 (signed)
        "t64": dram("t64", (4, P, T)),         # c64q s64q c64k s64k (dup halves)
        "tabm": dram("tabm", (6, P, T)),       # c2q s2q c2k s2k cBq sBq
        "vg": dram("vg", (4, T, 448)),         # va2 256 | va1 128 | vb1 64
        "consts": dram("consts", (2, P, P)),   # tri(NEG strict upper), iden
        "wvec": dram("wvec", (P, 4), dt=F32),
    }
    outQ = dram("outQ", (T, 1024), out=True)
    grp_r = [D["grp"][g].rearrange("c p t -> p c t") for g in range(4)]
    r1in_r = [D["r1in"][g].rearrange("c p t -> p c t") for g in range(4)]
    vg_r = [D["vg"][g].rearrange("(c p) d -> p c d", p=P) for g in range(4)]

    with ExitStack() as ctx:
        tc = ctx.enter_context(tile.TileContext(nc))
        pers = ctx.enter_context(tc.tile_pool(name="pers", bufs=1))
        pick = _Pick(nc)

        # ---------------- persistent tiles ----------------
        r1q = pers.tile([P, 4, T], BF, name="r1q")
        r1k = pers.tile([P, 4, T], BF, name="r1k")
        tabmA = pers.tile([P, 2, T], BF, name="tabmA")
        tabm = pers.tile([P, 4, T], BF, name="tabm")
        cst = pers.tile([P, 2, P], BF, name="cst")
        wv = pers.tile([P, 4], F32, name="wv")
        t64t = pers.tile([P, 4, T], BF, name="t64t")
        tri, iden = cst[:, 0, :], cst[:, 1, :]
        ones1 = pers.tile([P, 1], BF, name="ones1")
        nc.vector.memset(ones1, 1.0)

        tabr1q = pers.tile([P, 2, T], BF, name="tabr1q")
        tabr1k = pers.tile([P, 2, T], BF, name="tabr1k")

        # ---------------- work pools ----------------
        mixp = ctx.enter_context(tc.tile_pool(name="mix", bufs=2))
        scr = ctx.enter_context(tc.tile_pool(name="scr", bufs=2))
        ptp = ctx.enter_context(tc.tile_pool(name="pt", bufs=2))
        tAp = ctx.enter_context(tc.tile_pool(name="tA", bufs=2))
        outp = ctx.enter_context(tc.tile_pool(name="out", bufs=1))
        spsum = ctx.enter_context(tc.tile_pool(name="sp", bufs=2, space="PSUM"))
        ypsum = ctx.enter_context(tc.tile_pool(name="yp", bufs=4, space="PSUM"))

        c2q, s2q = tabmA[:, 0, :], tabmA[:, 1, :]
        c2k, s2k = tabm[:, 0, :], tabm[:, 1, :]
        cBq, sBq = tabm[:, 2, :], tabm[:, 3, :]

        state = {}

        def prefetch(g):
            if g >= 4 or ("grpq", g) in state:
                return
            r1gq = mixp.tile([P, 2, T], BF, tag="r1gq", name="r1gq", bufs=1)
            nc.sync.dma_start(out=r1gq, in_=r1in_r[g][:, 0:2, :])
            grpq = mixp.tile([P, 2, T], BF, tag="grpq", name="grpq")
            nc.sync.dma_start(out=grpq, in_=grp_r[g][:, 0:2, :])
            r1gk = mixp.tile([P, 2, T], BF, tag="r1gk", name="r1gk", bufs=1)
            nc.sync.dma_start(out=r1gk, in_=r1in_r[g][:, 2:4, :])
            grpk = mixp.tile([P, 2, T], BF, tag="grpk", name="grpk")
            nc.sync.dma_start(out=grpk, in_=grp_r[g][:, 2:4, :])
            grpd = mixp.tile([P, 3, T], BF, tag="grpd", name="grpd")
            nc.sync.dma_start(out=grpd, in_=grp_r[g][:, 4:7, :])
            vgt = mixp.tile([P, TK, 448], BF, tag="vg", name="vg")
            nc.sync.dma_start(out=vgt, in_=vg_r[g])
            state[("r1gq", g)] = r1gq
            state[("r1gk", g)] = r1gk
            state[("grpq", g)] = grpq
            state[("grpk", g)] = grpk
            state[("grpd", g)] = grpd
            state[("vg", g)] = vgt

        def r1build(g):
            r1gq, r1gk = state[("r1gq", g)], state[("r1gk", g)]
            u1 = scr.tile([P, T], BF, tag="u1", name="u1", bufs=1)
            pick.tt(u1, r1gq[:, 1, :], tabr1q[:, 1, :], mult, T)
            pick.tt(r1q[:, g, :], r1gq[:, 0, :], tabr1q[:, 0, :], mult, T)
            pick.tt(r1q[:, g, :], r1q[:, g, :], u1, add, T)
            u2 = scr.tile([P, T], BF, tag="u1b", name="u1b", bufs=1)
            pick.tt(u2, r1gk[:, 1, :], tabr1k[:, 1, :], mult, T)
            pick.tt(r1k[:, g, :], r1gk[:, 0, :], tabr1k[:, 0, :], mult, T)
            pick.tt(r1k[:, g, :], r1k[:, g, :], u2, add, T)

        # group-0 critical-path loads, consumption-ordered on the SP queue
        r1gq0 = mixp.tile([P, 2, T], BF, tag="r1gq", name="r1gq0", bufs=1)
        nc.sync.dma_start(out=r1gq0, in_=r1in_r[0][:, 0:2, :])
        nc.sync.dma_start(out=tabr1q,
                          in_=D["tabr1"].rearrange("c p t -> p c t")[:, 0:2, :])
        grpq0 = mixp.tile([P, 2, T], BF, tag="grpq", name="grpq0")
        nc.sync.dma_start(out=grpq0, in_=grp_r[0][:, 0:2, :])
        nc.sync.dma_start(out=tabmA,
                          in_=D["tabm"].rearrange("c p t -> p c t")[:, 0:2, :])
        r1gk0 = mixp.tile([P, 2, T], BF, tag="r1gk", name="r1gk0", bufs=1)
        nc.sync.dma_start(out=r1gk0, in_=r1in_r[0][:, 2:4, :])
        nc.sync.dma_start(out=tabr1k,
                          in_=D["tabr1"].rearrange("c p t -> p c t")[:, 2:4, :])
        grpk0 = mixp.tile([P, 2, T], BF, tag="grpk", name="grpk0")
        nc.sync.dma_start(out=grpk0, in_=grp_r[0][:, 2:4, :])
        nc.sync.dma_start(out=cst, in_=D["consts"].rearrange("c p t -> p c t"))
        nc.sync.dma_start(out=wv, in_=D["wvec"][:, :])
        nc.sync.dma_start(out=tabm,
                          in_=D["tabm"].rearrange("c p t -> p c t")[:, 2:6, :])
        nc.sync.dma_start(out=t64t, in_=D["t64"].rearrange("c p t -> p c t"))
        grpd0 = mixp.tile([P, 3, T], BF, tag="grpd", name="grpd0")
        nc.sync.dma_start(out=grpd0, in_=grp_r[0][:, 4:7, :])
        vg0 = mixp.tile([P, TK, 448], BF, tag="vg", name="vg0")
        nc.sync.dma_start(out=vg0, in_=vg_r[0])
        state[("r1gq", 0)] = r1gq0
        state[("r1gk", 0)] = r1gk0
        state[("grpq", 0)] = grpq0
        state[("grpk", 0)] = grpk0
        state[("grpd", 0)] = grpd0
        state[("vg", 0)] = vg0

        def mix_A(xt, ch, cpos, spos, r1, tag):
            """[P,2,T] mix for config-A (d=256 rope + folded d128 part)."""
            qm = mixp.tile([P, 2, T], BF, tag=tag, name=tag)
            u = scr.tile([P, T], BF, tag="uA", name="uA")
            x0, x1 = xt[:, ch, :], xt[:, ch + 1, :]
            pick.tt(u, x1, spos, mult, T)
            pick.tt(qm[:, 0, :], x0, cpos, mult, T)
            pick.tt(qm[:, 0, :], qm[:, 0, :], u, sub, T)
            pick.tt(qm[:, 0, :], qm[:, 0, :], r1, add, T)
            pick.tt(u, x0, spos, mult, T)
            pick.tt(qm[:, 1, :], x1, cpos, mult, T)
            pick.tt(qm[:, 1, :], qm[:, 1, :], u, add, T)
            return qm

        def r64build(src, ch, ctab, stab, dst, dch):
            """dst[0:64, dch] = rope64 of packed src channel ch (grp tile)."""
            u = scr.tile([P, T], BF, tag="u64", name="u64")
            pick.tt(dst[0:64, dch, :], src[0:64, ch, :], stab[0:64, :], mult, T)
            pick.tt(u[64:P, :], src[64:P, ch, :], ctab[64:P, :], mult, T)
            pick.cp(u[0:64, :], u[64:P, :], T)
            pick.tt(dst[0:64, dch, :], dst[0:64, dch, :], u[0:64, :], add, T)

        def phase1(qm_chunks, km_chunks, pt):
            ndc = len(qm_chunks)
            for c in range(TK):
                q0 = P * c
                sT = spsum.tile([P, T], F32, tag="sT", name="sT")
                pieces = ([(q0, 512), (512, T)] if c < 4 else [(q0, T)])
                for (a, b) in pieces:
                    for dc in range(ndc):
                        nc.tensor.matmul(sT[:, a:b],
                                         km_chunks[dc][:, q0:q0 + P],
                                         qm_chunks[dc][:, a:b],
                                         start=(dc == 0), stop=(dc == ndc - 1))
                nc.tensor.matmul(sT[:, q0:q0 + P], tri, iden,
                                 start=False, stop=True, skip_group_check=True)
                nc.scalar.activation(pt[:, c, q0:T], sT[:, q0:T], Exp)

        def phase2_A(pt, vm, tA):
            rec = scr.tile([P, TK], F32, tag="recA", name="recA")
            for qc in range(TK):
                y = ypsum.tile([P, 512], F32, tag="y", name="y")
                for c in range(qc + 1):
                    nc.tensor.matmul(y[:, 0:257],
                                     pt[:, c, P * qc:P * qc + P],
                                     vm[:, c, :],
                                     start=(c == 0), stop=(c == qc))
                nc.vector.reciprocal(rec[:, qc:qc + 1], y[:, 256:257])
                nc.scalar.activation(tA[:, qc, :], y[:, 0:256],
                                     mybir.ActivationFunctionType.Copy,
                                     scale=rec[:, qc:qc + 1])

        def phase2_B(pt, vm, tA, outt, hh, late=False):
            rec = scr.tile([P, TK], F32, tag="recB", name="recB")
            for qc in range(TK):
                y = ypsum.tile([P, 512], F32, tag="y", name="y")
                for c in range(qc + 1):
                    nc.tensor.matmul(y[:, 0:129],
                                     pt[:, c, P * qc:P * qc + P],
                                     vm[:, c, :],
                                     start=(c == 0), stop=(c == qc))
                nc.vector.reciprocal(rec[:, qc:qc + 1], y[:, 128:129])
                pick.stt(outt[:, qc, 128 * hh:128 * hh + 128],
                         y[:, 0:128], rec[:, qc:qc + 1],
                         tA[:, qc, 128 * hh:128 * hh + 128],
                         mult, add, 128, psum=True)

        def do_A(g):
            r1build(g)
            prefetch(g + 1)
            qm = mix_A(state[("grpq", g)], 0, c2q, s2q, r1q[:, g, :], "qmA")
            km = mix_A(state[("grpk", g)], 0, c2k, s2k, r1k[:, g, :], "kmA")
            vgt = state[("vg", g)]
            vm = mixp.tile([P, TK, 257], BF, tag="vmA", name="vmA", bufs=1)
            uv = scr.tile([P, TK, P], BF, tag="uvA", name="uvA")
            pick.tsp(vm[:, :, 0:256], vgt[:, :, 0:256], wv[:, 1:2], mult, 2048)
            pick.tsp(uv, vgt[:, :, 256:384], wv[:, 0:1], mult, 1024)
            pick.tt(vm[:, :, 0:P], vm[:, :, 0:P], uv, add, 1024)
            nc.vector.memset(vm[:, :, 256:257], 1.0)
            pt = ptp.tile([P, TK, T], BF, tag="pt", name="ptA")
            phase1([qm[:, 0, :], qm[:, 1, :]], [km[:, 0, :], km[:, 1, :]], pt)
            tA = tAp.tile([P, TK, 256], BF, tag="tA", name="tA")
            phase2_A(pt, vm, tA)
            state[g] = tA

        def do_B(h):
            g, hh = h // 2, h % 2
            grp = state[("grpq", g)]
            grpd = state[("grpd", g)]
            vgt = state[("vg", g)]
            if hh == 0:
                r64g = mixp.tile([64, 2, T], BF, tag="r64q", name="r64q")
                r64build(grpd, 0, t64t[:, 0, :], t64t[:, 1, :], r64g, 0)
                r64build(grpd, 1, t64t[:, 0, :], t64t[:, 1, :], r64g, 1)
                r64kg = mixp.tile([64, 1, T], BF, tag="r64k", name="r64k")
                r64build(grpd, 2, t64t[:, 2, :], t64t[:, 3, :], r64kg, 0)
                km = mixp.tile([P, T], BF, tag="kmB", name="kmB")
                pick.tt(km[0:64, :], r1k[0:64, g, :], r64kg[:, 0, :], add, T)
                pick.cp(km[64:P, :], r1k[64:P, g, :], T)
                vm = mixp.tile([P, TK, 129], BF, tag="vmB", name="vmB", bufs=1)
                uv = scr.tile([P, TK, 64], BF, tag="uvB", name="uvB")
                pick.tsp(vm[:, :, 0:128], vgt[:, :, 256:384], wv[:, 3:4], mult, 1024)
                pick.tsp(uv, vgt[:, :, 384:448], wv[:, 2:3], mult, 512)
                pick.tt(vm[:, :, 0:64], vm[:, :, 0:64], uv, add, 512)
                nc.vector.memset(vm[:, :, 128:129], 1.0)
                state[("B", g)] = (km, vm, r64g)
                outt = outp.tile([P, TK, 256], BF, tag="outt", name="outt")
                state[("o", g)] = outt
            km, vm, r64g = state[("B", g)]
            outt = state[("o", g)]
            qm = mixp.tile([P, T], BF, tag="qmB", name="qmB")
            u = scr.tile([P, T], BF, tag="uB", name="uB")
            sg = scr.tile([P, T], BF, tag="sgB", name="sgB")
            pick.cp(sg[0:64, :], grp[64:P, hh, :], T)
            pick.cp(sg[64:P, :], grp[0:64, hh, :], T)
            pick.tt(u, sg, sBq, mult, T)
            pick.tt(qm, grp[:, hh, :], cBq, mult, T)
            pick.tt(qm, qm, u, add, T)
            pick.tt(qm[0:64, :], qm[0:64, :], r64g[:, hh, :], add, T)
            pt = ptp.tile([P, TK, T], BF, tag="pt", name="ptB")
            phase1([qm], [km], pt)
            phase2_B(pt, vm, state[g], outt, hh, late=(g >= 2))
            if hh == 1:
                outr = outQ.rearrange("(c p) d -> p c d", p=P)
                nc.sync.dma_start(
                    out=outr[:, 0:4, 256 * g:256 * g + 256],
                    in_=outt[:, 0:4, :])
                nc.sync.dma_start(
                    out=outr[:, 4:8, 256 * g:256 * g + 256],
                    in_=outt[:, 4:8, :])

        for g in range(4):
            do_A(g)
            do_B(2 * g)
            do_B(2 * g + 1)

    nc.compile()
    return nc


# ---------------------------------------------------------------------------
# Host side
# ---------------------------------------------------------------------------

def _rope_tabs(pos, d, scale=1.0):
    """cos/sin tables [d, T]; sin SIGNED math-order (rows < d/2 negated)."""
    inv = 1.0 / (10000.0 ** (np.arange(0, d, 2, dtype=np.float32) / d))
    ang = inv[:, None] * pos[None, :].astype(np.float32)
    ang = np.concatenate([ang, ang], 0)
    c = (scale * np.cos(ang)).astype(np.float32)
    s = (scale * np.sin(ang)).astype(np.float32)
    s[: d // 2] *= -1.0
    return c, s


def _sigma(x, half):
    sh = x.shape
    y = x.reshape(-1, 2, half, *sh[1:])
    return np.ascontiguousarray(y[:, ::-1].reshape(sh))


def make_core_inputs(q, k, v, pos, weights, s, cfg: KCfg = FULL):
    """q,k,v: [T, 2048] fp32 for one batch; returns per-core input dict."""
    bf = lambda x: np.ascontiguousarray(x, dtype=NPBF)
    w0, w1, w2, w3 = [float(x) for x in weights]
    fA2 = 1.0 / 16.0
    fB2 = 1.0 / math.sqrt(128.0)

    qa1 = q[:, 512 * s:512 * s + 512].T          # [512, T]
    qa2 = q[:, 1024 * s:1024 * s + 1024].T       # [1024, T]
    ka1 = k[:, 512 * s:512 * s + 512].T
    ka2 = k[:, 1024 * s:1024 * s + 1024].T
    kb1 = k[:, 256 * s:256 * s + 256].T          # [256, T]

    qa1b = qa1.reshape(4, P, T)
    qa1s = _sigma(qa1, 64).reshape(4, P, T)
    ka1b = ka1.reshape(4, P, T)
    ka1s = _sigma(ka1, 64).reshape(4, P, T)

    c1q, s1q = _rope_tabs(pos, 128, fA2 * w0 * w0)
    c1, s1 = _rope_tabs(pos, 128)
    tabr1 = np.stack([c1q, s1q, c1, s1])

    # packed d64: rows 0:64 sigma32 data, rows 64:128 raw data
    dq = qa1.reshape(8, 64, T)
    dqs = _sigma(qa1, 32).reshape(8, 64, T)
    d64q = np.concatenate([dqs, dq], 1)                       # [8, 128, T]
    dk = kb1.reshape(4, 64, T)
    dks = _sigma(kb1, 32).reshape(4, 64, T)
    d64k = np.concatenate([dks, dk], 1)                       # [4, 128, T]
    qa2b = qa2.reshape(8, P, T)
    ka2b = ka2.reshape(8, P, T)
    r1in = np.stack([np.concatenate([
        qa1b[g:g + 1], qa1s[g:g + 1], ka1b[g:g + 1], ka1s[g:g + 1]], 0)
        for g in range(4)])
    grp = np.stack([np.concatenate([
        qa2b[2 * g:2 * g + 2], ka2b[2 * g:2 * g + 2],
        d64q[2 * g:2 * g + 2], d64k[g:g + 1]], 0) for g in range(4)])

    c64q, s64q = _rope_tabs(pos, 64, fB2 * w3 * w2)
    c64k, s64k = _rope_tabs(pos, 64, w2 / w3)
    t64 = np.stack([np.concatenate([c64q, c64q], 0),
                    np.concatenate([s64q, s64q], 0),
                    np.concatenate([c64k, c64k], 0),
                    np.concatenate([s64k, s64k], 0)])         # [4, 128, T]

    c2q, s2q = _rope_tabs(pos, 256, fA2 * w0 * w1)
    c2k, s2k = _rope_tabs(pos, 256, w1 / w0)
    cBq, sBq = _rope_tabs(pos, 128, fB2 * w3 * w3)
    tabm = np.stack([c2q[:P], -s2q[:P], c2k[:P], -s2k[:P], cBq, sBq])

    va1 = v[:, 512 * s:512 * s + 512]
    va2 = v[:, 1024 * s:1024 * s + 1024]
    vb1 = v[:, 256 * s:256 * s + 256]
    vg = np.stack([np.concatenate([
        va2[:, 256 * g:256 * g + 256], va1[:, 128 * g:128 * g + 128],
        vb1[:, 64 * g:64 * g + 64]], 1) for g in range(4)])   # [4, T, 448]

    tri = np.zeros((P, P), np.float32)
    j, kk = np.mgrid[0:P, 0:P]
    tri[j < kk] = NEG
    consts = np.stack([tri, np.eye(P, dtype=np.float32)])

    arrs = {
        "grp": bf(grp), "r1in": bf(r1in), "tabr1": bf(tabr1), "t64": bf(t64),
        "tabm": bf(tabm), "vg": bf(vg), "consts": bf(consts),
        "wvec": np.tile(np.asarray(weights, np.float32)[None, :], (P, 1)),
    }
    return arrs


_PROGRAM_CACHE = {}
TRACE = False
LAST_RESULT = None


def kernel(q_m, k_m, v_m, weights, attention_mask, position_ids):
    global LAST_RESULT
    from concourse.bass_utils import run_bass_kernel_spmd

    cfg = FULL
    q_m = np.asarray(q_m, np.float32)
    k_m = np.asarray(k_m, np.float32)
    v_m = np.asarray(v_m, np.float32)
    weights = np.asarray(weights, np.float32)
    attention_mask = np.asarray(attention_mask, np.float32)
    position_ids = np.asarray(position_ids)
    B, Tq, H = q_m.shape

    causal = np.where(np.tril(np.ones((Tq, Tq), bool)), 0.0, NEG).astype(np.float32)
    for b in range(B):
        assert np.array_equal(attention_mask[b, 0], causal), "non-causal mask"

    if "nc" not in _PROGRAM_CACHE:
        _PROGRAM_CACHE["nc"] = build_program(cfg)
    nc = _PROGRAM_CACHE["nc"]

    in_maps = []
    for b in range(B):
        for s in range(2):
            in_maps.append(make_core_inputs(
                q_m[b], k_m[b], v_m[b], position_ids[b], weights, s, cfg))
    res = run_bass_kernel_spmd(nc, in_maps, list(range(8)), trace=TRACE)
    LAST_RESULT = res
    out = np.zeros((B, Tq, H), np.float32)
    for b in range(B):
        for s in range(2):
            out[b, :, 1024 * s:1024 * s + 1024] = \
                res.results[2 * b + s]["outQ"].astype(np.float32)
    return out
